# revision 49
# baseline (speedup 1.0000x reference)
"""MiniChessNNUE kernel for 8 Trainium2 NeuronCores.

Data-parallel: batch (16384) sharded 2048/core, weights replicated.

Math (per core, batch slice n):
  w_acc = screlu(white @ ft_w.T + ft_b)      [n, 128]
  b_acc = screlu(black @ ft_w.T + ft_b)      [n, 128]
  x     = concat(where(stm, b_acc, w_acc), where(stm, w_acc, b_acc))
  z1    = x @ l1_w.T + l1_b
With l1_w = [A | B] the select folds into matmul algebra:
  z1 = A@w' + B@b' + (A-B)@(stm * (b'-w'))   (w'/b' = screlu'd accums)
Everything on device is computed transposed ([feature, batch] layout) so the
contraction dim sits on SBUF partitions and every bias becomes a K=1 rank-1
matmul against a ones row.

Perf-critical layout facts (measured on hw):
  * A DMA whose SBUF side spans 125 partitions is striped over only 5 of the
    16 SDMA engines (~130 GB/s); 128 partitions engage all 16 (~390 GB/s).
    So the contraction dim is zero-padded 9000 -> 9216 = 72*128.
  * The two perspective streams ride separate HWDGE rings (SP + ACT) and are
    consumed interleaved tile-by-tile so both rings stay busy and pool
    buffers recycle evenly.
  * f16 everywhere on device: half the HBM traffic of f32, full PE rate,
    and ~1.4e-3 rel error (fp8 would be ~5e-2 -- gated out).
"""

import os

import numpy as np

import concourse.bass as bass  # noqa: F401
import concourse.tile as tile
from concourse import bacc, mybir
from concourse.bass_utils import run_bass_kernel_spmd

# Containers without the full antenv package lack the axon NTFF hook module
# that run_bass_kernel_spmd imports when BASS_TRACE is set; stub it so trace
# requests degrade to "no trace" instead of crashing.
try:
    from antenv import axon_hooks as _axon_hooks  # noqa: F401
except ImportError:
    import sys
    import types

    _m = types.ModuleType("antenv.axon_hooks")
    _m.get_axon_ntff_profile_hook = lambda: None
    sys.modules["antenv.axon_hooks"] = _m

N_CORES = 8
B = 16384
F = 9000
FP = 9216                # F zero-padded to 72 * 128 (pad rows contribute 0)
ACC = 128
L1 = 32
L2 = 32

BC = B // N_CORES        # 2048 batch rows per core
KP = 128                 # contraction partitions per chunk (see header)
NK = FP // KP            # 72 k-chunks
K_FULL = F // KP         # 70 fully-populated k-chunks
K_PART = F - K_FULL * KP  # 40 live rows in k-chunk 70; chunk 71 is all pad
NFREE = 512              # matmul moving free dim (f32 PSUM bank = 512 cols;
                         # larger breaks the NEFF compile)
NCH = BC // NFREE        # 4 batch chunks ("stages") per core
MFREE = 512              # MLP-tail moving free dim

KO = int(os.environ.get("NNUE_KO", "8"))          # k-chunks per DMA tile
FEAT_BUFS = int(os.environ.get("NNUE_FEAT_BUFS", "6"))
# 1 = features ship as uint8 and the gpsimd SWDGE cast-DMA decodes to f16
# inline (q in 0..255, ft_w/255 folded on host). Halves HBM reads; the DMA
# engines are dst-byte-bound so this trims the HBM-cap slack. 0 = f16 direct.
U8_FEATS = int(os.environ.get("NNUE_U8", "1"))
# Of the 9 tiles per stage-perspective, this many ship raw u8 on the (idle)
# sync HWDGE ring (1 B/elem of DMA dst) and are decoded u8->f16 by the DVE /
# scalar engines; the rest go through the gpsimd cast-DMA (2 B/elem dst).
# Decode capacity: DVE ~215 G elem/s, scalar ~136 G elem/s (measured).
RAW_TILES = int(os.environ.get("NNUE_RAW", "8"))

F16 = mybir.dt.float16
F32 = mybir.dt.float32
U8 = mybir.dt.uint8

LAST_RESULT = None  # BassKernelResults of the most recent run (for profiling)


def _build(ko: int = KO, feat_bufs: int = FEAT_BUFS, u8: bool = bool(U8_FEATS)):
    fdt = F16
    sdt = U8 if u8 else F16  # feature dtype in HBM
    nt = NK // ko

    nc = bacc.Bacc("TRN2", target_bir_lowering=False, debug=False)
    # features are host pre-tiled into the exact DMA order: tile st*nt+t is
    # one contiguous [KP, ko, NFREE] block (max-efficiency HBM reads)
    wT = nc.dram_tensor("wT", [NCH * nt, KP, ko, NFREE], sdt,
                        kind="ExternalInput")
    bT = nc.dram_tensor("bT", [NCH * nt, KP, ko, NFREE], sdt,
                        kind="ExternalInput")
    # host pre-permuted so partition p holds rows {k*128+p} contiguously
    ftwT = nc.dram_tensor("ftwT", [KP, NK, ACC], fdt, kind="ExternalInput")
    ftb = nc.dram_tensor("ftb", [1, ACC], fdt, kind="ExternalInput")
    smask = nc.dram_tensor("smask", [ACC, BC], fdt, kind="ExternalInput")
    l1A = nc.dram_tensor("l1A", [ACC, L1], fdt, kind="ExternalInput")
    l1B = nc.dram_tensor("l1B", [ACC, L1], fdt, kind="ExternalInput")
    l1D = nc.dram_tensor("l1D", [ACC, L1], fdt, kind="ExternalInput")
    l1b = nc.dram_tensor("l1b", [1, L1], fdt, kind="ExternalInput")
    l2wT = nc.dram_tensor("l2wT", [L1, L2], fdt, kind="ExternalInput")
    l2b = nc.dram_tensor("l2b", [1, L2], fdt, kind="ExternalInput")
    owT = nc.dram_tensor("owT", [L2, 1], fdt, kind="ExternalInput")
    ob = nc.dram_tensor("ob", [1, 1], fdt, kind="ExternalInput")
    y = nc.dram_tensor("y", [1, BC], F32, kind="ExternalOutput")

    raw = RAW_TILES if u8 else 0
    lead = int(os.environ.get("NNUE_LEAD", "0")) if u8 else 0
    if lead:
        # first tiles of stage 0 ship as ready-to-use f16 (same integer
        # values the u8 decode would produce) so the PE starts immediately
        # while the decode pipeline spins up
        wL = nc.dram_tensor("wL", [lead, KP, ko, NFREE], F16,
                            kind="ExternalInput")
        bL = nc.dram_tensor("bL", [lead, KP, ko, NFREE], F16,
                            kind="ExternalInput")

    with tile.TileContext(nc) as tc:
        with (
            tc.tile_pool(name="consts", bufs=1) as consts,
            tc.tile_pool(name="feat", bufs=feat_bufs) as featp,
            tc.tile_pool(name="raw8", bufs=6) as rawp,
            tc.tile_pool(name="acts", bufs=2) as actp,
            tc.tile_pool(name="psum_ft", bufs=2, space="PSUM") as psum_ft,
            tc.tile_pool(name="psum_s", bufs=1, space="PSUM") as psum_s,
        ):
            # ftw loaded in per-tile chunks so the first matmuls don't wait
            # for the whole 2.4 MB; chunk t's DMA is issued just before tile
            # t's feature DMAs during stage 0 (see below)
            ftw_ck = [consts.tile([KP, ko, ACC], fdt, tag=f"ftw{t}",
                                  name=f"ftw_{t}")
                      for t in range(nt)]
            ones_sb = consts.tile([1, NFREE], fdt)
            nc.vector.memset(ones_sb[:], 1.0)
            ftb_sb = consts.tile([1, ACC], fdt)
            nc.scalar.dma_start(ftb_sb[:], ftb[:])
            smask_sb = consts.tile([ACC, BC], fdt)
            # gpsimd ring: not needed until the first stage tail (~50 us in)
            nc.gpsimd.dma_start(smask_sb[:], smask[:])
            l1A_sb = consts.tile([ACC, L1], fdt)
            nc.scalar.dma_start(l1A_sb[:], l1A[:])
            l1B_sb = consts.tile([ACC, L1], fdt)
            nc.scalar.dma_start(l1B_sb[:], l1B[:])
            l1D_sb = consts.tile([ACC, L1], fdt)
            nc.scalar.dma_start(l1D_sb[:], l1D[:])
            l1b_sb = consts.tile([1, L1], fdt)
            nc.scalar.dma_start(l1b_sb[:], l1b[:])
            l2wT_sb = consts.tile([L1, L2], fdt)
            nc.scalar.dma_start(l2wT_sb[:], l2wT[:])
            l2b_sb = consts.tile([1, L2], fdt)
            nc.scalar.dma_start(l2b_sb[:], l2b[:])
            owT_sb = consts.tile([L2, 1], fdt)
            nc.scalar.dma_start(owT_sb[:], owT[:])
            ob_sb = consts.tile([1, 1], fdt)
            nc.scalar.dma_start(ob_sb[:], ob[:])

            y_sb = consts.tile([1, BC], F32)

            clamp01 = (0.0, 1.0, mybir.AluOpType.max, mybir.AluOpType.min)

            def emit_screlu(st, accW, accB):
                """screlu both perspectives of a finished stage (clamp on
                DVE, square on scalar); returns the sq tiles."""
                sq = []
                for pi, acc in ((0, accW), (1, accB)):
                    s = actp.tile([ACC, NFREE], fdt, tag=f"sq{pi}",
                                  name=f"sq{pi}_{st}")
                    nc.vector.tensor_scalar(s[:], acc[:], *clamp01)
                    nc.scalar.square(s[:], s[:])
                    sq.append(s)
                return sq

            def emit_tail(st, sq):
                """stm-select + MLP head for a finished stage.

                Called a few tiles INTO the next stage (screlu earlier, this
                later) so the burst of DVE/scalar work is spread out and
                doesn't starve the decode pipeline at the stage boundary."""
                c0 = st * NFREE
                # d = stm * (b' - w') -- on the (idle) gpsimd engine so the
                # tail burst doesn't stall the DVE's decode stream
                d_sb = actp.tile([ACC, NFREE], fdt, tag="d", name=f"d_{st}")
                nc.gpsimd.tensor_sub(out=d_sb[:], in0=sq[1][:], in1=sq[0][:])
                nc.gpsimd.tensor_mul(out=d_sb[:], in0=d_sb[:],
                                     in1=smask_sb[:, c0:c0 + NFREE])

                for n in range(NFREE // MFREE):
                    ns = slice(n * MFREE, (n + 1) * MFREE)
                    ones_h = ones_sb[:, :MFREE]
                    p1 = psum_s.tile([L1, MFREE], F32, tag="p1",
                                     name=f"p1_{st}_{n}")
                    nc.tensor.matmul(p1[:], l1A_sb[:], sq[0][:, ns],
                                     start=True, stop=False)
                    nc.tensor.matmul(p1[:], l1B_sb[:], sq[1][:, ns],
                                     start=False, stop=False)
                    nc.tensor.matmul(p1[:], l1D_sb[:], d_sb[:, ns],
                                     start=False, stop=False)
                    nc.tensor.matmul(p1[:], l1b_sb[:], ones_h,
                                     start=False, stop=True)
                    h1 = actp.tile([L1, MFREE], fdt, tag="h1",
                                   name=f"h1_{st}_{n}")
                    nc.vector.tensor_scalar(h1[:], p1[:], *clamp01)
                    nc.scalar.square(h1[:], h1[:])

                    p2 = psum_s.tile([L2, MFREE], F32, tag="p2",
                                     name=f"p2_{st}_{n}")
                    nc.tensor.matmul(p2[:], l2wT_sb[:], h1[:],
                                     start=True, stop=False)
                    nc.tensor.matmul(p2[:], l2b_sb[:], ones_h,
                                     start=False, stop=True)
                    h2 = actp.tile([L2, MFREE], fdt, tag="h2",
                                   name=f"h2_{st}_{n}")
                    nc.vector.tensor_scalar(h2[:], p2[:], *clamp01)
                    nc.scalar.square(h2[:], h2[:])

                    p3 = psum_s.tile([1, MFREE], F32, tag="p3",
                                     name=f"p3_{st}_{n}")
                    nc.tensor.matmul(p3[:], owT_sb[:], h2[:],
                                     start=True, stop=False)
                    nc.tensor.matmul(p3[:], ob_sb[:], ones_h,
                                     start=False, stop=True)
                    nc.vector.tensor_copy(
                        out=y_sb[:, c0 + n * MFREE:c0 + (n + 1) * MFREE],
                        in_=p3[:])

            # only ftw chunk 0 ahead of the first feature tiles; the rest
            # interleave with stage-0 tile DMAs (prefetch distance 1) so
            # pair 0 starts moving as early as possible
            nc.sync.dma_start(ftw_ck[0][:], ftwT[:, :ko, :])

            pending_tail = None
            pending_sq = None
            for st in range(NCH):
                accW = psum_ft.tile([ACC, NFREE], F32, tag="accW",
                                    name=f"accW_{st}")
                accB = psum_ft.tile([ACC, NFREE], F32, tag="accB",
                                    name=f"accB_{st}")
                # bias as rank-1 update opens each accumulation group
                nc.tensor.matmul(accW[:], ftb_sb[:], ones_sb[:],
                                 start=True, stop=False)
                nc.tensor.matmul(accB[:], ftb_sb[:], ones_sb[:],
                                 start=True, stop=False)
                for t in range(nt):
                    # live k-chunks in this tile: all pad (k >= 70.3) skipped
                    # across DMA, decode and matmul
                    kfull = min(ko, max(0, K_FULL - t * ko))  # full chunks
                    part = K_FULL < (t + 1) * ko              # has the 40-row
                    klive = kfull + (1 if part else 0)
                    if t == 2 and pending_tail is not None:
                        pending_sq = (pending_tail[0],
                                      emit_screlu(*pending_tail))
                        pending_tail = None
                    if t == 5 and pending_sq is not None:
                        emit_tail(*pending_sq)
                        pending_sq = None
                    wt = featp.tile([KP, ko, NFREE], fdt, tag="fw",
                                    name=f"fw_{st}_{t}")
                    bt = featp.tile([KP, ko, NFREE], fdt, tag="fb",
                                    name=f"fb_{st}_{t}")
                    use_raw = (t * raw) % nt < raw

                    def load(f16t, dram, eng):
                        """DMA the live region of one tile (trimming the pad
                        rows of the partial chunk and all-dead chunks)."""
                        src = dram[st * nt + t]
                        eng.dma_start(f16t[:, :kfull, :], src[:, :kfull, :])
                        if part:
                            eng.dma_start(f16t[:K_PART, kfull, :],
                                          src[:K_PART, kfull, :])

                    if u8 and st == 0 and t < lead:
                        nc.sync.dma_start(wt[:], wL[t])
                        nc.scalar.dma_start(bt[:], bL[t])
                    elif u8 and use_raw:
                        # raw u8 over the idle sync ring, decoded u8->f16.
                        # Each tile splits across both decode engines --
                        # scalar (~136 G elem/s) takes ~3 of 8 k-chunks,
                        # DVE (~215 G/s) the rest -- so per-tile latency
                        # stays low and loads balance by construction.
                        # The first pairs' b-tile rides the (empty) scalar
                        # ring so the pipeline fills on two rings at once.
                        ks = 3 * ko // 8
                        for pers, (f16t, dram) in enumerate(
                                ((wt, wT), (bt, bT))):
                            t8 = rawp.tile([KP, ko, NFREE], U8,
                                           tag=f"r{pers}",
                                           name=f"r{pers}_{st}_{t}")
                            ring = (nc.scalar if pers and st == 0 and t < 3
                                    else nc.sync)
                            load(t8, dram, ring)
                            nc.scalar.copy(f16t[:, :ks, :], t8[:, :ks, :])
                            nc.vector.tensor_copy(
                                out=f16t[:, ks:kfull, :],
                                in_=t8[:, ks:kfull, :])
                            if part:
                                nc.vector.tensor_copy(
                                    out=f16t[:K_PART, kfull, :],
                                    in_=t8[:K_PART, kfull, :])
                    elif u8:
                        # SWDGE cast-DMA decodes uint8 -> f16 inline
                        load(wt, wT, nc.gpsimd)
                        load(bt, bT, nc.gpsimd)
                    else:
                        load(wt, wT, nc.sync)
                        load(bt, bT, nc.scalar)
                    if st == 0 and t + 1 < nt:
                        t1 = t + 1
                        kl1 = (min(ko, max(0, K_FULL - t1 * ko))
                               + (1 if K_FULL < (t1 + 1) * ko else 0))
                        nc.sync.dma_start(
                            ftw_ck[t1][:, :kl1, :],
                            ftwT[:, t1 * ko:t1 * ko + kl1, :])
                    for kk in range(klive):
                        k = t * ko + kk
                        rows = slice(None) if kk < kfull else slice(0, K_PART)
                        last = k == K_FULL
                        nc.tensor.matmul(accW[:], ftw_ck[t][rows, kk, :],
                                         wt[rows, kk, :], start=False,
                                         stop=last)
                        nc.tensor.matmul(accB[:], ftw_ck[t][rows, kk, :],
                                         bt[rows, kk, :], start=False,
                                         stop=last)

                pending_tail = (st, accW, accB)

            emit_tail(pending_tail[0], emit_screlu(*pending_tail))
            nc.sync.dma_start(y[:], y_sb[:])

    nc.compile()
    return nc


_NC_CACHE: dict = {}


def _pretile(arr, dtype, ko=None):
    """[BC, F] feature slice -> zero-padded, transposed, device-DMA-ordered
    [NCH*NT, KP, ko, NFREE] so each feature tile is one contiguous HBM
    block."""
    ko = KO if ko is None else ko
    nt = NK // ko
    arr_T = np.zeros((FP, BC), dtype=dtype)
    arr_T[:F] = arr.T  # cast + transpose + zero-pad in one pass
    return np.ascontiguousarray(
        arr_T.reshape(nt, ko, KP, NCH, NFREE)
             .transpose(3, 0, 2, 1, 4)
             .reshape(NCH * nt, KP, ko, NFREE))


def kernel(white_features, black_features, stm, ft_w, ft_b,
           l1_w, l1_b, l2_w, l2_b, out_w, out_b) -> np.ndarray:
    global LAST_RESULT
    f16 = np.float16

    white_features = np.asarray(white_features)
    black_features = np.asarray(black_features)
    stm = np.asarray(stm)
    ft_w = np.asarray(ft_w, dtype=np.float32)
    ft_b = np.asarray(ft_b, dtype=np.float32)
    l1_w = np.asarray(l1_w, dtype=np.float32)
    l1_b = np.asarray(l1_b, dtype=np.float32)
    l2_w = np.asarray(l2_w, dtype=np.float32)
    l2_b = np.asarray(l2_b, dtype=np.float32)
    out_w = np.asarray(out_w, dtype=np.float32)
    out_b = np.asarray(out_b, dtype=np.float32)

    u8 = bool(U8_FEATS)
    if u8:
        # features quantize to q = rint(f * 255) in uint8 (exact fixed point,
        # decoded by the cast-DMA as integer-valued f16); fold /255 into ft_w
        white_q = np.rint(white_features * np.float32(255.0)).astype(np.uint8)
        black_q = np.rint(black_features * np.float32(255.0)).astype(np.uint8)
        ftw_eff = ft_w * np.float32(1.0 / 255.0)
        feat_src_w, feat_src_b, src_np = white_q, black_q, np.uint8
    else:
        ftw_eff = ft_w
        feat_src_w, feat_src_b, src_np = white_features, black_features, f16

    # [F, 128] zero-padded to [FP, 128] -> [128, 72, 128] with
    # [p, k, m] = ft_w.T[k*128+p, m], so the device DMA is one
    # fully-contiguous read
    ftw_pad = np.zeros((FP, ACC), dtype=f16)
    ftw_pad[:F] = ftw_eff.T
    ftwT = np.ascontiguousarray(
        ftw_pad.reshape(NK, KP, ACC).transpose(1, 0, 2))
    A = l1_w[:, :ACC]
    Bm = l1_w[:, ACC:]
    shared = {
        "ftwT": ftwT,
        "ftb": ft_b[None, :].astype(f16),                        # [1, 128]
        "l1A": np.ascontiguousarray(A.T).astype(f16),            # [128, 32]
        "l1B": np.ascontiguousarray(Bm.T).astype(f16),
        "l1D": np.ascontiguousarray((A - Bm).T).astype(f16),
        "l1b": l1_b[None, :].astype(f16),
        "l2wT": np.ascontiguousarray(l2_w.T).astype(f16),        # [32, 32]
        "l2b": l2_b[None, :].astype(f16),
        "owT": np.ascontiguousarray(out_w.T).astype(f16),        # [32, 1]
        "ob": out_b[None, :].astype(f16),                        # [1, 1]
    }

    stm_f = stm.astype(np.float32)
    in_maps = []
    for c in range(N_CORES):
        sl = slice(c * BC, (c + 1) * BC)
        wTc = _pretile(feat_src_w[sl], src_np)
        bTc = _pretile(feat_src_b[sl], src_np)
        im = {
            "wT": wTc,
            "bT": bTc,
            "smask": np.ascontiguousarray(
                np.broadcast_to(stm_f[sl][None, :], (ACC, BC))).astype(f16),
            **shared,
        }
        lead = int(os.environ.get("NNUE_LEAD", "0")) if u8 else 0
        if lead:
            im["wL"] = wTc[:lead].astype(f16)
            im["bL"] = bTc[:lead].astype(f16)
        in_maps.append(im)

    if "nc" not in _NC_CACHE:
        _NC_CACHE["nc"] = _build()
    nc = _NC_CACHE["nc"]

    LAST_RESULT = run_bass_kernel_spmd(nc, in_maps, core_ids=list(range(N_CORES)))
    out = np.concatenate(
        [LAST_RESULT.results[c]["y"].reshape(BC) for c in range(N_CORES)])
    return out.astype(np.float32)


# revision 50
# speedup vs baseline: 1.0119x; 1.0119x over previous
"""MiniChessNNUE kernel for 8 Trainium2 NeuronCores.

Data-parallel: batch (16384) sharded 2048/core, weights replicated.

Math (per core, batch slice n):
  w_acc = screlu(white @ ft_w.T + ft_b)      [n, 128]
  b_acc = screlu(black @ ft_w.T + ft_b)      [n, 128]
  x     = concat(where(stm, b_acc, w_acc), where(stm, w_acc, b_acc))
  z1    = x @ l1_w.T + l1_b
With l1_w = [A | B] the select folds into matmul algebra:
  z1 = A@w' + B@b' + (A-B)@(stm * (b'-w'))   (w'/b' = screlu'd accums)
Everything on device is computed transposed ([feature, batch] layout) so the
contraction dim sits on SBUF partitions and every bias becomes a K=1 rank-1
matmul against a ones row.

Perf-critical layout facts (measured on hw):
  * A DMA whose SBUF side spans 125 partitions is striped over only 5 of the
    16 SDMA engines (~130 GB/s); 128 partitions engage all 16 (~390 GB/s).
    So the contraction dim is zero-padded 9000 -> 9216 = 72*128.
  * The two perspective streams ride separate HWDGE rings (SP + ACT) and are
    consumed interleaved tile-by-tile so both rings stay busy and pool
    buffers recycle evenly.
  * f16 everywhere on device: half the HBM traffic of f32, full PE rate,
    and ~1.4e-3 rel error (fp8 would be ~5e-2 -- gated out).
"""

import os

import numpy as np

import concourse.bass as bass  # noqa: F401
import concourse.tile as tile
from concourse import bacc, mybir
from concourse.bass_utils import run_bass_kernel_spmd

# Containers without the full antenv package lack the axon NTFF hook module
# that run_bass_kernel_spmd imports when BASS_TRACE is set; stub it so trace
# requests degrade to "no trace" instead of crashing.
try:
    from antenv import axon_hooks as _axon_hooks  # noqa: F401
except ImportError:
    import sys
    import types

    _m = types.ModuleType("antenv.axon_hooks")
    _m.get_axon_ntff_profile_hook = lambda: None
    sys.modules["antenv.axon_hooks"] = _m

N_CORES = 8
B = 16384
F = 9000
FP = 9216                # F zero-padded to 72 * 128 (pad rows contribute 0)
ACC = 128
L1 = 32
L2 = 32

BC = B // N_CORES        # 2048 batch rows per core
KP = 128                 # contraction partitions per chunk (see header)
NK = FP // KP            # 72 k-chunks
K_FULL = F // KP         # 70 fully-populated k-chunks
K_PART = F - K_FULL * KP  # 40 live rows in k-chunk 70; chunk 71 is all pad
NFREE = 512              # matmul moving free dim (f32 PSUM bank = 512 cols;
                         # larger breaks the NEFF compile)
NCH = BC // NFREE        # 4 batch chunks ("stages") per core
MFREE = 512              # MLP-tail moving free dim

KO = int(os.environ.get("NNUE_KO", "8"))          # k-chunks per DMA tile
FEAT_BUFS = int(os.environ.get("NNUE_FEAT_BUFS", "6"))
# 1 = features ship as uint8 and the gpsimd SWDGE cast-DMA decodes to f16
# inline (q in 0..255, ft_w/255 folded on host). Halves HBM reads; the DMA
# engines are dst-byte-bound so this trims the HBM-cap slack. 0 = f16 direct.
U8_FEATS = int(os.environ.get("NNUE_U8", "1"))
# Of the 9 tiles per stage-perspective, this many ship raw u8 on the (idle)
# sync HWDGE ring (1 B/elem of DMA dst) and are decoded u8->f16 by the DVE /
# scalar engines; the rest go through the gpsimd cast-DMA (2 B/elem dst).
# Decode capacity: DVE ~215 G elem/s, scalar ~136 G elem/s (measured).
RAW_TILES = int(os.environ.get("NNUE_RAW", "8"))

F16 = mybir.dt.float16
F32 = mybir.dt.float32
U8 = mybir.dt.uint8

LAST_RESULT = None  # BassKernelResults of the most recent run (for profiling)


def _build(ko: int = KO, feat_bufs: int = FEAT_BUFS, u8: bool = bool(U8_FEATS)):
    fdt = F16
    sdt = U8 if u8 else F16  # feature dtype in HBM
    nt = NK // ko

    nc = bacc.Bacc("TRN2", target_bir_lowering=False, debug=False)
    # features are host pre-tiled into the exact DMA order: tile st*nt+t is
    # one contiguous [KP, ko, NFREE] block (max-efficiency HBM reads)
    wT = nc.dram_tensor("wT", [NCH * nt, KP, ko, NFREE], sdt,
                        kind="ExternalInput")
    bT = nc.dram_tensor("bT", [NCH * nt, KP, ko, NFREE], sdt,
                        kind="ExternalInput")
    # host pre-permuted so partition p holds rows {k*128+p} contiguously
    ftwT = nc.dram_tensor("ftwT", [KP, NK, ACC], fdt, kind="ExternalInput")
    ftb = nc.dram_tensor("ftb", [1, ACC], fdt, kind="ExternalInput")
    smask = nc.dram_tensor("smask", [ACC, BC], fdt, kind="ExternalInput")
    l1A = nc.dram_tensor("l1A", [ACC, L1], fdt, kind="ExternalInput")
    l1B = nc.dram_tensor("l1B", [ACC, L1], fdt, kind="ExternalInput")
    l1D = nc.dram_tensor("l1D", [ACC, L1], fdt, kind="ExternalInput")
    l1b = nc.dram_tensor("l1b", [1, L1], fdt, kind="ExternalInput")
    l2wT = nc.dram_tensor("l2wT", [L1, L2], fdt, kind="ExternalInput")
    l2b = nc.dram_tensor("l2b", [1, L2], fdt, kind="ExternalInput")
    owT = nc.dram_tensor("owT", [L2, 1], fdt, kind="ExternalInput")
    ob = nc.dram_tensor("ob", [1, 1], fdt, kind="ExternalInput")
    y = nc.dram_tensor("y", [1, BC], F32, kind="ExternalOutput")

    raw = RAW_TILES if u8 else 0
    lead = int(os.environ.get("NNUE_LEAD", "0")) if u8 else 0
    if lead:
        # first tiles of stage 0 ship as ready-to-use f16 (same integer
        # values the u8 decode would produce) so the PE starts immediately
        # while the decode pipeline spins up
        wL = nc.dram_tensor("wL", [lead, KP, ko, NFREE], F16,
                            kind="ExternalInput")
        bL = nc.dram_tensor("bL", [lead, KP, ko, NFREE], F16,
                            kind="ExternalInput")

    with tile.TileContext(nc) as tc:
        with (
            tc.tile_pool(name="consts", bufs=1) as consts,
            tc.tile_pool(name="feat", bufs=feat_bufs) as featp,
            tc.tile_pool(name="raw8", bufs=6) as rawp,
            tc.tile_pool(name="acts", bufs=2) as actp,
            tc.tile_pool(name="psum_ft", bufs=2, space="PSUM") as psum_ft,
            tc.tile_pool(name="psum_s", bufs=1, space="PSUM") as psum_s,
        ):
            # ftw loaded in per-tile chunks so the first matmuls don't wait
            # for the whole 2.4 MB; chunk t's DMA is issued just before tile
            # t's feature DMAs during stage 0 (see below)
            ftw_ck = [consts.tile([KP, ko, ACC], fdt, tag=f"ftw{t}",
                                  name=f"ftw_{t}")
                      for t in range(nt)]
            ones_sb = consts.tile([1, NFREE], fdt)
            nc.vector.memset(ones_sb[:], 1.0)
            ftb_sb = consts.tile([1, ACC], fdt)
            nc.scalar.dma_start(ftb_sb[:], ftb[:])
            smask_sb = consts.tile([ACC, BC], fdt)
            # gpsimd ring: not needed until the first stage tail (~50 us in)
            nc.gpsimd.dma_start(smask_sb[:], smask[:])
            l1A_sb = consts.tile([ACC, L1], fdt)
            nc.scalar.dma_start(l1A_sb[:], l1A[:])
            l1B_sb = consts.tile([ACC, L1], fdt)
            nc.scalar.dma_start(l1B_sb[:], l1B[:])
            l1D_sb = consts.tile([ACC, L1], fdt)
            nc.scalar.dma_start(l1D_sb[:], l1D[:])
            l1b_sb = consts.tile([1, L1], fdt)
            nc.scalar.dma_start(l1b_sb[:], l1b[:])
            l2wT_sb = consts.tile([L1, L2], fdt)
            nc.scalar.dma_start(l2wT_sb[:], l2wT[:])
            l2b_sb = consts.tile([1, L2], fdt)
            nc.scalar.dma_start(l2b_sb[:], l2b[:])
            owT_sb = consts.tile([L2, 1], fdt)
            nc.scalar.dma_start(owT_sb[:], owT[:])
            ob_sb = consts.tile([1, 1], fdt)
            nc.scalar.dma_start(ob_sb[:], ob[:])

            y_sb = consts.tile([1, BC], F32)

            clamp01 = (0.0, 1.0, mybir.AluOpType.max, mybir.AluOpType.min)

            def emit_tail(st, accW, accB):
                """screlu + stm-select + MLP head for a finished stage.

                Called a few tiles INTO the next stage so this burst of
                DVE/scalar work doesn't starve the decode pipeline at the
                stage boundary."""
                c0 = st * NFREE
                sq = []
                for pi, acc in ((0, accW), (1, accB)):
                    s = actp.tile([ACC, NFREE], fdt, tag=f"sq{pi}",
                                  name=f"sq{pi}_{st}")
                    nc.vector.tensor_scalar(s[:], acc[:], *clamp01)
                    nc.scalar.square(s[:], s[:])
                    sq.append(s)

                # d = stm * (b' - w') -- on the (idle) gpsimd engine so the
                # tail burst doesn't stall the DVE's decode stream
                d_sb = actp.tile([ACC, NFREE], fdt, tag="d", name=f"d_{st}")
                nc.gpsimd.tensor_sub(out=d_sb[:], in0=sq[1][:], in1=sq[0][:])
                nc.gpsimd.tensor_mul(out=d_sb[:], in0=d_sb[:],
                                     in1=smask_sb[:, c0:c0 + NFREE])

                for n in range(NFREE // MFREE):
                    ns = slice(n * MFREE, (n + 1) * MFREE)
                    ones_h = ones_sb[:, :MFREE]
                    p1 = psum_s.tile([L1, MFREE], F32, tag="p1",
                                     name=f"p1_{st}_{n}")
                    nc.tensor.matmul(p1[:], l1A_sb[:], sq[0][:, ns],
                                     start=True, stop=False)
                    nc.tensor.matmul(p1[:], l1B_sb[:], sq[1][:, ns],
                                     start=False, stop=False)
                    nc.tensor.matmul(p1[:], l1D_sb[:], d_sb[:, ns],
                                     start=False, stop=False)
                    nc.tensor.matmul(p1[:], l1b_sb[:], ones_h,
                                     start=False, stop=True)
                    h1 = actp.tile([L1, MFREE], fdt, tag="h1",
                                   name=f"h1_{st}_{n}")
                    nc.vector.tensor_scalar(h1[:], p1[:], *clamp01)
                    nc.vector.tensor_mul(out=h1[:], in0=h1[:], in1=h1[:])

                    p2 = psum_s.tile([L2, MFREE], F32, tag="p2",
                                     name=f"p2_{st}_{n}")
                    nc.tensor.matmul(p2[:], l2wT_sb[:], h1[:],
                                     start=True, stop=False)
                    nc.tensor.matmul(p2[:], l2b_sb[:], ones_h,
                                     start=False, stop=True)
                    h2 = actp.tile([L2, MFREE], fdt, tag="h2",
                                   name=f"h2_{st}_{n}")
                    nc.vector.tensor_scalar(h2[:], p2[:], *clamp01)
                    nc.vector.tensor_mul(out=h2[:], in0=h2[:], in1=h2[:])

                    p3 = psum_s.tile([1, MFREE], F32, tag="p3",
                                     name=f"p3_{st}_{n}")
                    nc.tensor.matmul(p3[:], owT_sb[:], h2[:],
                                     start=True, stop=False)
                    nc.tensor.matmul(p3[:], ob_sb[:], ones_h,
                                     start=False, stop=True)
                    nc.vector.tensor_copy(
                        out=y_sb[:, c0 + n * MFREE:c0 + (n + 1) * MFREE],
                        in_=p3[:])

            # only ftw chunk 0 ahead of the first feature tiles; the rest
            # interleave with stage-0 tile DMAs (prefetch distance 1) so
            # pair 0 starts moving as early as possible
            nc.sync.dma_start(ftw_ck[0][:], ftwT[:, :ko, :])

            pending_tail = None
            for st in range(NCH):
                accW = psum_ft.tile([ACC, NFREE], F32, tag="accW",
                                    name=f"accW_{st}")
                accB = psum_ft.tile([ACC, NFREE], F32, tag="accB",
                                    name=f"accB_{st}")
                # bias as rank-1 update opens each accumulation group
                nc.tensor.matmul(accW[:], ftb_sb[:], ones_sb[:],
                                 start=True, stop=False)
                nc.tensor.matmul(accB[:], ftb_sb[:], ones_sb[:],
                                 start=True, stop=False)
                for t in range(nt):
                    # live k-chunks in this tile: all pad (k >= 70.3) skipped
                    # across DMA, decode and matmul
                    kfull = min(ko, max(0, K_FULL - t * ko))  # full chunks
                    part = K_FULL < (t + 1) * ko              # has the 40-row
                    klive = kfull + (1 if part else 0)
                    if t == 3 and pending_tail is not None:
                        emit_tail(*pending_tail)
                        pending_tail = None
                    wt = featp.tile([KP, ko, NFREE], fdt, tag="fw",
                                    name=f"fw_{st}_{t}")
                    bt = featp.tile([KP, ko, NFREE], fdt, tag="fb",
                                    name=f"fb_{st}_{t}")
                    use_raw = (t * raw) % nt < raw

                    def load(f16t, dram, eng):
                        """DMA the live region of one tile (trimming the pad
                        rows of the partial chunk and all-dead chunks)."""
                        src = dram[st * nt + t]
                        eng.dma_start(f16t[:, :kfull, :], src[:, :kfull, :])
                        if part:
                            eng.dma_start(f16t[:K_PART, kfull, :],
                                          src[:K_PART, kfull, :])

                    if u8 and st == 0 and t < lead:
                        nc.sync.dma_start(wt[:], wL[t])
                        nc.scalar.dma_start(bt[:], bL[t])
                    elif u8 and use_raw:
                        # raw u8 over the idle sync ring, decoded u8->f16.
                        # Each tile splits across both decode engines --
                        # scalar (~136 G elem/s) takes ~3 of 8 k-chunks,
                        # DVE (~215 G/s) the rest -- so per-tile latency
                        # stays low and loads balance by construction.
                        # The first pairs' b-tile rides the (empty) scalar
                        # ring so the pipeline fills on two rings at once.
                        ks = 3 * ko // 8
                        for pers, (f16t, dram) in enumerate(
                                ((wt, wT), (bt, bT))):
                            t8 = rawp.tile([KP, ko, NFREE], U8,
                                           tag=f"r{pers}",
                                           name=f"r{pers}_{st}_{t}")
                            ring = (nc.scalar if pers and st == 0 and t < 3
                                    else nc.sync)
                            load(t8, dram, ring)
                            nc.scalar.copy(f16t[:, :ks, :], t8[:, :ks, :])
                            nc.vector.tensor_copy(
                                out=f16t[:, ks:kfull, :],
                                in_=t8[:, ks:kfull, :])
                            if part:
                                nc.vector.tensor_copy(
                                    out=f16t[:K_PART, kfull, :],
                                    in_=t8[:K_PART, kfull, :])
                    elif u8:
                        # SWDGE cast-DMA decodes uint8 -> f16 inline
                        load(wt, wT, nc.gpsimd)
                        load(bt, bT, nc.gpsimd)
                    else:
                        load(wt, wT, nc.sync)
                        load(bt, bT, nc.scalar)
                    if st == 0 and t + 1 < nt:
                        t1 = t + 1
                        kl1 = (min(ko, max(0, K_FULL - t1 * ko))
                               + (1 if K_FULL < (t1 + 1) * ko else 0))
                        nc.sync.dma_start(
                            ftw_ck[t1][:, :kl1, :],
                            ftwT[:, t1 * ko:t1 * ko + kl1, :])
                    for kk in range(klive):
                        k = t * ko + kk
                        rows = slice(None) if kk < kfull else slice(0, K_PART)
                        last = k == K_FULL
                        nc.tensor.matmul(accW[:], ftw_ck[t][rows, kk, :],
                                         wt[rows, kk, :], start=False,
                                         stop=last)
                        nc.tensor.matmul(accB[:], ftw_ck[t][rows, kk, :],
                                         bt[rows, kk, :], start=False,
                                         stop=last)

                pending_tail = (st, accW, accB)

            emit_tail(*pending_tail)
            nc.sync.dma_start(y[:], y_sb[:])

    nc.compile()
    return nc


_NC_CACHE: dict = {}


def _pretile(arr, dtype, ko=None):
    """[BC, F] feature slice -> zero-padded, transposed, device-DMA-ordered
    [NCH*NT, KP, ko, NFREE] so each feature tile is one contiguous HBM
    block."""
    ko = KO if ko is None else ko
    nt = NK // ko
    arr_T = np.zeros((FP, BC), dtype=dtype)
    arr_T[:F] = arr.T  # cast + transpose + zero-pad in one pass
    return np.ascontiguousarray(
        arr_T.reshape(nt, ko, KP, NCH, NFREE)
             .transpose(3, 0, 2, 1, 4)
             .reshape(NCH * nt, KP, ko, NFREE))


def kernel(white_features, black_features, stm, ft_w, ft_b,
           l1_w, l1_b, l2_w, l2_b, out_w, out_b) -> np.ndarray:
    global LAST_RESULT
    f16 = np.float16

    white_features = np.asarray(white_features)
    black_features = np.asarray(black_features)
    stm = np.asarray(stm)
    ft_w = np.asarray(ft_w, dtype=np.float32)
    ft_b = np.asarray(ft_b, dtype=np.float32)
    l1_w = np.asarray(l1_w, dtype=np.float32)
    l1_b = np.asarray(l1_b, dtype=np.float32)
    l2_w = np.asarray(l2_w, dtype=np.float32)
    l2_b = np.asarray(l2_b, dtype=np.float32)
    out_w = np.asarray(out_w, dtype=np.float32)
    out_b = np.asarray(out_b, dtype=np.float32)

    u8 = bool(U8_FEATS)
    if u8:
        # features quantize to q = rint(f * 255) in uint8 (exact fixed point,
        # decoded by the cast-DMA as integer-valued f16); fold /255 into ft_w
        white_q = np.rint(white_features * np.float32(255.0)).astype(np.uint8)
        black_q = np.rint(black_features * np.float32(255.0)).astype(np.uint8)
        ftw_eff = ft_w * np.float32(1.0 / 255.0)
        feat_src_w, feat_src_b, src_np = white_q, black_q, np.uint8
    else:
        ftw_eff = ft_w
        feat_src_w, feat_src_b, src_np = white_features, black_features, f16

    # [F, 128] zero-padded to [FP, 128] -> [128, 72, 128] with
    # [p, k, m] = ft_w.T[k*128+p, m], so the device DMA is one
    # fully-contiguous read
    ftw_pad = np.zeros((FP, ACC), dtype=f16)
    ftw_pad[:F] = ftw_eff.T
    ftwT = np.ascontiguousarray(
        ftw_pad.reshape(NK, KP, ACC).transpose(1, 0, 2))
    A = l1_w[:, :ACC]
    Bm = l1_w[:, ACC:]
    shared = {
        "ftwT": ftwT,
        "ftb": ft_b[None, :].astype(f16),                        # [1, 128]
        "l1A": np.ascontiguousarray(A.T).astype(f16),            # [128, 32]
        "l1B": np.ascontiguousarray(Bm.T).astype(f16),
        "l1D": np.ascontiguousarray((A - Bm).T).astype(f16),
        "l1b": l1_b[None, :].astype(f16),
        "l2wT": np.ascontiguousarray(l2_w.T).astype(f16),        # [32, 32]
        "l2b": l2_b[None, :].astype(f16),
        "owT": np.ascontiguousarray(out_w.T).astype(f16),        # [32, 1]
        "ob": out_b[None, :].astype(f16),                        # [1, 1]
    }

    stm_f = stm.astype(np.float32)
    in_maps = []
    for c in range(N_CORES):
        sl = slice(c * BC, (c + 1) * BC)
        wTc = _pretile(feat_src_w[sl], src_np)
        bTc = _pretile(feat_src_b[sl], src_np)
        im = {
            "wT": wTc,
            "bT": bTc,
            "smask": np.ascontiguousarray(
                np.broadcast_to(stm_f[sl][None, :], (ACC, BC))).astype(f16),
            **shared,
        }
        lead = int(os.environ.get("NNUE_LEAD", "0")) if u8 else 0
        if lead:
            im["wL"] = wTc[:lead].astype(f16)
            im["bL"] = bTc[:lead].astype(f16)
        in_maps.append(im)

    if "nc" not in _NC_CACHE:
        _NC_CACHE["nc"] = _build()
    nc = _NC_CACHE["nc"]

    LAST_RESULT = run_bass_kernel_spmd(nc, in_maps, core_ids=list(range(N_CORES)))
    out = np.concatenate(
        [LAST_RESULT.results[c]["y"].reshape(BC) for c in range(N_CORES)])
    return out.astype(np.float32)


# revision 54
# speedup vs baseline: 1.0152x; 1.0032x over previous
"""MiniChessNNUE kernel for 8 Trainium2 NeuronCores.

Data-parallel: batch (16384) sharded 2048/core, weights replicated.

Math (per core, batch slice n):
  w_acc = screlu(white @ ft_w.T + ft_b)      [n, 128]
  b_acc = screlu(black @ ft_w.T + ft_b)      [n, 128]
  x     = concat(where(stm, b_acc, w_acc), where(stm, w_acc, b_acc))
  z1    = x @ l1_w.T + l1_b
With l1_w = [A | B] the select folds into matmul algebra:
  z1 = A@w' + B@b' + (A-B)@(stm * (b'-w'))   (w'/b' = screlu'd accums)
Everything on device is computed transposed ([feature, batch] layout) so the
contraction dim sits on SBUF partitions and every bias becomes a K=1 rank-1
matmul against a ones row.

Perf-critical layout facts (measured on hw):
  * A DMA whose SBUF side spans 125 partitions is striped over only 5 of the
    16 SDMA engines (~130 GB/s); 128 partitions engage all 16 (~390 GB/s).
    So the contraction dim is zero-padded 9000 -> 9216 = 72*128.
  * The two perspective streams ride separate HWDGE rings (SP + ACT) and are
    consumed interleaved tile-by-tile so both rings stay busy and pool
    buffers recycle evenly.
  * f16 everywhere on device: half the HBM traffic of f32, full PE rate,
    and ~1.4e-3 rel error (fp8 would be ~5e-2 -- gated out).
"""

import os

import numpy as np

import concourse.bass as bass  # noqa: F401
import concourse.tile as tile
from concourse import bacc, mybir
from concourse.bass_utils import run_bass_kernel_spmd

# Containers without the full antenv package lack the axon NTFF hook module
# that run_bass_kernel_spmd imports when BASS_TRACE is set; stub it so trace
# requests degrade to "no trace" instead of crashing.
try:
    from antenv import axon_hooks as _axon_hooks  # noqa: F401
except ImportError:
    import sys
    import types

    _m = types.ModuleType("antenv.axon_hooks")
    _m.get_axon_ntff_profile_hook = lambda: None
    sys.modules["antenv.axon_hooks"] = _m

N_CORES = 8
B = 16384
F = 9000
FP = 9216                # F zero-padded to 72 * 128 (pad rows contribute 0)
ACC = 128
L1 = 32
L2 = 32

BC = B // N_CORES        # 2048 batch rows per core
KP = 128                 # contraction partitions per chunk (see header)
NK = FP // KP            # 72 k-chunks
K_FULL = F // KP         # 70 fully-populated k-chunks
K_PART = F - K_FULL * KP  # 40 live rows in k-chunk 70; chunk 71 is all pad
NFREE = 512              # matmul moving free dim (f32 PSUM bank = 512 cols;
                         # larger breaks the NEFF compile)
NCH = BC // NFREE        # 4 batch chunks ("stages") per core
MFREE = 512              # MLP-tail moving free dim

KO = int(os.environ.get("NNUE_KO", "8"))          # k-chunks per DMA tile
FEAT_BUFS = int(os.environ.get("NNUE_FEAT_BUFS", "6"))
# 1 = features ship as uint8 and the gpsimd SWDGE cast-DMA decodes to f16
# inline (q in 0..255, ft_w/255 folded on host). Halves HBM reads; the DMA
# engines are dst-byte-bound so this trims the HBM-cap slack. 0 = f16 direct.
U8_FEATS = int(os.environ.get("NNUE_U8", "1"))
# Of the 9 tiles per stage-perspective, this many ship raw u8 on the (idle)
# sync HWDGE ring (1 B/elem of DMA dst) and are decoded u8->f16 by the DVE /
# scalar engines; the rest go through the gpsimd cast-DMA (2 B/elem dst).
# Decode capacity: DVE ~215 G elem/s, scalar ~136 G elem/s (measured).
RAW_TILES = int(os.environ.get("NNUE_RAW", "8"))

F16 = mybir.dt.float16
F32 = mybir.dt.float32
U8 = mybir.dt.uint8

LAST_RESULT = None  # BassKernelResults of the most recent run (for profiling)


def _build(ko: int = KO, feat_bufs: int = FEAT_BUFS, u8: bool = bool(U8_FEATS)):
    fdt = F16
    sdt = U8 if u8 else F16  # feature dtype in HBM
    nt = NK // ko

    nc = bacc.Bacc("TRN2", target_bir_lowering=False, debug=False)
    # features are host pre-tiled into the exact DMA order: tile st*nt+t is
    # one contiguous [KP, ko, NFREE] block (max-efficiency HBM reads)
    wT = nc.dram_tensor("wT", [NCH * nt, KP, ko, NFREE], sdt,
                        kind="ExternalInput")
    bT = nc.dram_tensor("bT", [NCH * nt, KP, ko, NFREE], sdt,
                        kind="ExternalInput")
    # host pre-permuted so partition p holds rows {k*128+p} contiguously
    ftwT = nc.dram_tensor("ftwT", [KP, NK, ACC], fdt, kind="ExternalInput")
    ftb = nc.dram_tensor("ftb", [1, ACC], fdt, kind="ExternalInput")
    smask = nc.dram_tensor("smask", [ACC, BC], fdt, kind="ExternalInput")
    l1A = nc.dram_tensor("l1A", [ACC, L1], fdt, kind="ExternalInput")
    l1B = nc.dram_tensor("l1B", [ACC, L1], fdt, kind="ExternalInput")
    l1D = nc.dram_tensor("l1D", [ACC, L1], fdt, kind="ExternalInput")
    l1b = nc.dram_tensor("l1b", [1, L1], fdt, kind="ExternalInput")
    l2wT = nc.dram_tensor("l2wT", [L1, L2], fdt, kind="ExternalInput")
    l2b = nc.dram_tensor("l2b", [1, L2], fdt, kind="ExternalInput")
    owT = nc.dram_tensor("owT", [L2, 1], fdt, kind="ExternalInput")
    ob = nc.dram_tensor("ob", [1, 1], fdt, kind="ExternalInput")
    y = nc.dram_tensor("y", [1, BC], F32, kind="ExternalOutput")

    raw = RAW_TILES if u8 else 0
    lead = int(os.environ.get("NNUE_LEAD", "0")) if u8 else 0
    if lead:
        # first tiles of stage 0 ship as ready-to-use f16 (same integer
        # values the u8 decode would produce) so the PE starts immediately
        # while the decode pipeline spins up
        wL = nc.dram_tensor("wL", [lead, KP, ko, NFREE], F16,
                            kind="ExternalInput")
        bL = nc.dram_tensor("bL", [lead, KP, ko, NFREE], F16,
                            kind="ExternalInput")

    with tile.TileContext(nc) as tc:
        with (
            tc.tile_pool(name="consts", bufs=1) as consts,
            tc.tile_pool(name="feat", bufs=feat_bufs) as featp,
            tc.tile_pool(name="raw8", bufs=6) as rawp,
            tc.tile_pool(name="acts", bufs=2) as actp,
            tc.tile_pool(name="psum_ft", bufs=2, space="PSUM") as psum_ft,
            tc.tile_pool(name="psum_s", bufs=1, space="PSUM") as psum_s,
        ):
            # ftw loaded in per-tile chunks so the first matmuls don't wait
            # for the whole 2.4 MB; chunk t's DMA is issued just before tile
            # t's feature DMAs during stage 0 (see below)
            ftw_ck = [consts.tile([KP, ko, ACC], fdt, tag=f"ftw{t}",
                                  name=f"ftw_{t}")
                      for t in range(nt)]
            ones_sb = consts.tile([1, NFREE], fdt)
            nc.vector.memset(ones_sb[:], 1.0)
            ftb_sb = consts.tile([1, ACC], fdt)
            nc.scalar.dma_start(ftb_sb[:], ftb[:])
            smask_sb = consts.tile([ACC, BC], fdt)
            # gpsimd ring: not needed until the first stage tail (~50 us in)
            nc.gpsimd.dma_start(smask_sb[:], smask[:])
            l1A_sb = consts.tile([ACC, L1], fdt)
            nc.scalar.dma_start(l1A_sb[:], l1A[:])
            l1B_sb = consts.tile([ACC, L1], fdt)
            nc.scalar.dma_start(l1B_sb[:], l1B[:])
            l1D_sb = consts.tile([ACC, L1], fdt)
            nc.scalar.dma_start(l1D_sb[:], l1D[:])
            l1b_sb = consts.tile([1, L1], fdt)
            nc.scalar.dma_start(l1b_sb[:], l1b[:])
            l2wT_sb = consts.tile([L1, L2], fdt)
            nc.scalar.dma_start(l2wT_sb[:], l2wT[:])
            l2b_sb = consts.tile([1, L2], fdt)
            nc.scalar.dma_start(l2b_sb[:], l2b[:])
            owT_sb = consts.tile([L2, 1], fdt)
            nc.scalar.dma_start(owT_sb[:], owT[:])
            ob_sb = consts.tile([1, 1], fdt)
            nc.scalar.dma_start(ob_sb[:], ob[:])

            y_sb = consts.tile([1, BC], F32)

            clamp01 = (0.0, 1.0, mybir.AluOpType.max, mybir.AluOpType.min)

            def emit_tail(st, accW, accB):
                """screlu + stm-select + MLP head for a finished stage.

                Called a few tiles INTO the next stage so this burst of
                DVE/scalar work doesn't starve the decode pipeline at the
                stage boundary."""
                c0 = st * NFREE
                sq = []
                for pi, acc in ((0, accW), (1, accB)):
                    s = actp.tile([ACC, NFREE], fdt, tag=f"sq{pi}",
                                  name=f"sq{pi}_{st}")
                    nc.vector.tensor_scalar(s[:], acc[:], *clamp01)
                    nc.scalar.square(s[:], s[:])
                    sq.append(s)

                # d = stm * (b' - w') -- on the (idle) gpsimd engine so the
                # tail burst doesn't stall the DVE's decode stream
                d_sb = actp.tile([ACC, NFREE], fdt, tag="d", name=f"d_{st}")
                nc.gpsimd.tensor_sub(out=d_sb[:], in0=sq[1][:], in1=sq[0][:])
                nc.gpsimd.tensor_mul(out=d_sb[:], in0=d_sb[:],
                                     in1=smask_sb[:, c0:c0 + NFREE])

                for n in range(NFREE // MFREE):
                    ns = slice(n * MFREE, (n + 1) * MFREE)
                    ones_h = ones_sb[:, :MFREE]
                    p1 = psum_s.tile([L1, MFREE], F32, tag="p1",
                                     name=f"p1_{st}_{n}")
                    nc.tensor.matmul(p1[:], l1A_sb[:], sq[0][:, ns],
                                     start=True, stop=False)
                    nc.tensor.matmul(p1[:], l1B_sb[:], sq[1][:, ns],
                                     start=False, stop=False)
                    nc.tensor.matmul(p1[:], l1D_sb[:], d_sb[:, ns],
                                     start=False, stop=False)
                    nc.tensor.matmul(p1[:], l1b_sb[:], ones_h,
                                     start=False, stop=True)
                    h1 = actp.tile([L1, MFREE], fdt, tag="h1",
                                   name=f"h1_{st}_{n}")
                    nc.vector.tensor_scalar(h1[:], p1[:], *clamp01)
                    nc.vector.tensor_mul(out=h1[:], in0=h1[:], in1=h1[:])

                    p2 = psum_s.tile([L2, MFREE], F32, tag="p2",
                                     name=f"p2_{st}_{n}")
                    nc.tensor.matmul(p2[:], l2wT_sb[:], h1[:],
                                     start=True, stop=False)
                    nc.tensor.matmul(p2[:], l2b_sb[:], ones_h,
                                     start=False, stop=True)
                    h2 = actp.tile([L2, MFREE], fdt, tag="h2",
                                   name=f"h2_{st}_{n}")
                    nc.vector.tensor_scalar(h2[:], p2[:], *clamp01)
                    nc.vector.tensor_mul(out=h2[:], in0=h2[:], in1=h2[:])

                    p3 = psum_s.tile([1, MFREE], F32, tag="p3",
                                     name=f"p3_{st}_{n}")
                    nc.tensor.matmul(p3[:], owT_sb[:], h2[:],
                                     start=True, stop=False)
                    nc.tensor.matmul(p3[:], ob_sb[:], ones_h,
                                     start=False, stop=True)
                    nc.vector.tensor_copy(
                        out=y_sb[:, c0 + n * MFREE:c0 + (n + 1) * MFREE],
                        in_=p3[:])

            # only ftw chunk 0 ahead of the first feature tiles; the rest
            # interleave with stage-0 tile DMAs (prefetch distance 1) so
            # pair 0 starts moving as early as possible
            nc.sync.dma_start(ftw_ck[0][:], ftwT[:, :ko, :])

            pending_tail = None
            for st in range(NCH):
                accW = psum_ft.tile([ACC, NFREE], F32, tag="accW",
                                    name=f"accW_{st}")
                accB = psum_ft.tile([ACC, NFREE], F32, tag="accB",
                                    name=f"accB_{st}")
                # bias as rank-1 update opens each accumulation group
                nc.tensor.matmul(accW[:], ftb_sb[:], ones_sb[:],
                                 start=True, stop=False)
                nc.tensor.matmul(accB[:], ftb_sb[:], ones_sb[:],
                                 start=True, stop=False)
                for t in range(nt):
                    # live k-chunks in this tile: all pad (k >= 70.3) skipped
                    # across DMA, decode and matmul
                    kfull = min(ko, max(0, K_FULL - t * ko))  # full chunks
                    part = K_FULL < (t + 1) * ko              # has the 40-row
                    klive = kfull + (1 if part else 0)
                    if t == 3 and pending_tail is not None:
                        emit_tail(*pending_tail)
                        pending_tail = None
                    wt = featp.tile([KP, ko, NFREE], fdt, tag="fw",
                                    name=f"fw_{st}_{t}")
                    bt = featp.tile([KP, ko, NFREE], fdt, tag="fb",
                                    name=f"fb_{st}_{t}")
                    use_raw = (t * raw) % nt < raw

                    def load(f16t, dram, eng):
                        """DMA the live region of one tile (trimming the pad
                        rows of the partial chunk and all-dead chunks)."""
                        src = dram[st * nt + t]
                        eng.dma_start(f16t[:, :kfull, :], src[:, :kfull, :])
                        if part:
                            eng.dma_start(f16t[:K_PART, kfull, :],
                                          src[:K_PART, kfull, :])

                    if u8 and st == 0 and t < lead:
                        nc.sync.dma_start(wt[:], wL[t])
                        nc.scalar.dma_start(bt[:], bL[t])
                    elif u8 and use_raw:
                        # raw u8 over the idle sync ring, decoded u8->f16.
                        # Each tile splits across both decode engines --
                        # scalar (~136 G elem/s) takes ~3 of 8 k-chunks,
                        # DVE (~215 G/s) the rest -- so per-tile latency
                        # stays low and loads balance by construction.
                        # The first pairs' b-tile rides the (empty) scalar
                        # ring so the pipeline fills on two rings at once.
                        ks = 3 * ko // 8
                        for pers, (f16t, dram) in enumerate(
                                ((wt, wT), (bt, bT))):
                            t8 = rawp.tile([KP, ko, NFREE], U8,
                                           tag=f"r{pers}",
                                           name=f"r{pers}_{st}_{t}")
                            ring = (nc.scalar if pers and st == 0 and t < 3
                                    else nc.sync)
                            load(t8, dram, ring)
                            nc.scalar.copy(f16t[:, :ks, :], t8[:, :ks, :])
                            nc.vector.tensor_copy(
                                out=f16t[:, ks:kfull, :],
                                in_=t8[:, ks:kfull, :])
                            if part:
                                nc.vector.tensor_copy(
                                    out=f16t[:K_PART, kfull, :],
                                    in_=t8[:K_PART, kfull, :])
                    elif u8:
                        # SWDGE cast-DMA decodes uint8 -> f16 inline
                        load(wt, wT, nc.gpsimd)
                        load(bt, bT, nc.gpsimd)
                    else:
                        load(wt, wT, nc.sync)
                        load(bt, bT, nc.scalar)
                    if st == 0 and t + 1 < nt:
                        t1 = t + 1
                        kl1 = (min(ko, max(0, K_FULL - t1 * ko))
                               + (1 if K_FULL < (t1 + 1) * ko else 0))
                        nc.sync.dma_start(
                            ftw_ck[t1][:, :kl1, :],
                            ftwT[:, t1 * ko:t1 * ko + kl1, :])
                    for kk in range(klive):
                        k = t * ko + kk
                        rows = slice(None) if kk < kfull else slice(0, K_PART)
                        last = k == K_FULL
                        nc.tensor.matmul(accW[:], ftw_ck[t][rows, kk, :],
                                         wt[rows, kk, :], start=False,
                                         stop=last)
                        nc.tensor.matmul(accB[:], ftw_ck[t][rows, kk, :],
                                         bt[rows, kk, :], start=False,
                                         stop=last)

                pending_tail = (st, accW, accB)

            emit_tail(*pending_tail)
            nc.sync.dma_start(y[:], y_sb[:])

    nc.compile()
    return nc


_NC_CACHE: dict = {}


def _pretile(arr, dtype, ko=None):
    """[BC, F] feature slice -> zero-padded, transposed, device-DMA-ordered
    [NCH*NT, KP, ko, NFREE] so each feature tile is one contiguous HBM
    block."""
    ko = KO if ko is None else ko
    nt = NK // ko
    arr_T = np.zeros((FP, BC), dtype=dtype)
    arr_T[:F] = arr.T  # cast + transpose + zero-pad in one pass
    return np.ascontiguousarray(
        arr_T.reshape(nt, ko, KP, NCH, NFREE)
             .transpose(3, 0, 2, 1, 4)
             .reshape(NCH * nt, KP, ko, NFREE))


def kernel(white_features, black_features, stm, ft_w, ft_b,
           l1_w, l1_b, l2_w, l2_b, out_w, out_b) -> np.ndarray:
    global LAST_RESULT
    f16 = np.float16

    white_features = np.asarray(white_features)
    black_features = np.asarray(black_features)
    stm = np.asarray(stm)
    ft_w = np.asarray(ft_w, dtype=np.float32)
    ft_b = np.asarray(ft_b, dtype=np.float32)
    l1_w = np.asarray(l1_w, dtype=np.float32)
    l1_b = np.asarray(l1_b, dtype=np.float32)
    l2_w = np.asarray(l2_w, dtype=np.float32)
    l2_b = np.asarray(l2_b, dtype=np.float32)
    out_w = np.asarray(out_w, dtype=np.float32)
    out_b = np.asarray(out_b, dtype=np.float32)

    u8 = bool(U8_FEATS)
    if u8:
        # features quantize to q = rint(f * 255) in uint8 (exact fixed point,
        # decoded by the cast-DMA as integer-valued f16); fold /255 into ft_w
        white_q = np.rint(white_features * np.float32(255.0)).astype(np.uint8)
        black_q = np.rint(black_features * np.float32(255.0)).astype(np.uint8)
        ftw_eff = ft_w * np.float32(1.0 / 255.0)
        feat_src_w, feat_src_b, src_np = white_q, black_q, np.uint8
    else:
        ftw_eff = ft_w
        feat_src_w, feat_src_b, src_np = white_features, black_features, f16

    # [F, 128] zero-padded to [FP, 128] -> [128, 72, 128] with
    # [p, k, m] = ft_w.T[k*128+p, m], so the device DMA is one
    # fully-contiguous read
    ftw_pad = np.zeros((FP, ACC), dtype=f16)
    ftw_pad[:F] = ftw_eff.T
    ftwT = np.ascontiguousarray(
        ftw_pad.reshape(NK, KP, ACC).transpose(1, 0, 2))
    A = l1_w[:, :ACC]
    Bm = l1_w[:, ACC:]
    shared = {
        "ftwT": ftwT,
        "ftb": ft_b[None, :].astype(f16),                        # [1, 128]
        "l1A": np.ascontiguousarray(A.T).astype(f16),            # [128, 32]
        "l1B": np.ascontiguousarray(Bm.T).astype(f16),
        "l1D": np.ascontiguousarray((A - Bm).T).astype(f16),
        "l1b": l1_b[None, :].astype(f16),
        "l2wT": np.ascontiguousarray(l2_w.T).astype(f16),        # [32, 32]
        "l2b": l2_b[None, :].astype(f16),
        "owT": np.ascontiguousarray(out_w.T).astype(f16),        # [32, 1]
        "ob": out_b[None, :].astype(f16),                        # [1, 1]
    }

    stm_f = stm.astype(np.float32)
    in_maps = []
    for c in range(N_CORES):
        sl = slice(c * BC, (c + 1) * BC)
        wTc = _pretile(feat_src_w[sl], src_np)
        bTc = _pretile(feat_src_b[sl], src_np)
        im = {
            "wT": wTc,
            "bT": bTc,
            "smask": np.ascontiguousarray(
                np.broadcast_to(stm_f[sl][None, :], (ACC, BC))).astype(f16),
            **shared,
        }
        lead = int(os.environ.get("NNUE_LEAD", "0")) if u8 else 0
        if lead:
            im["wL"] = wTc[:lead].astype(f16)
            im["bL"] = bTc[:lead].astype(f16)
        in_maps.append(im)

    if "nc" not in _NC_CACHE:
        _NC_CACHE["nc"] = _build()
    nc = _NC_CACHE["nc"]

    LAST_RESULT = run_bass_kernel_spmd(nc, in_maps, core_ids=list(range(N_CORES)))
    out = np.concatenate(
        [LAST_RESULT.results[c]["y"].reshape(BC) for c in range(N_CORES)])
    return out.astype(np.float32)
